# revision 11
# baseline (speedup 1.0000x reference)
"""PointConditioner Trainium2 Bass kernel.

Per-core (1 batch element) pipeline, data-parallel over B=8 across 8 cores:
  1. conv1x1 as transposed matmuls: lhsT = P pixel-chunk [C=128, 128 pix],
     rhs = W^T [C=128, 64] -> PSUM [128 pix, 64 ch]; bias fused into the
     PSUM->SBUF copy (scalar_tensor_tensor add); written to a DRAM table
     T[HW+2, 64] with one guard row at each end (row r holds pixel r-1).
  2. bilinear sample via dma_gather of PAIRS: elem = 128 floats = rows
     (idx, idx+1) = x-taps (x0, x0+1) of one row y; two gathers (yA, yB).
     idx = clamp(y)*W + floor(x)+1, guard offset folded in.
  3. weights with zero-padding validity folded in; combine on DVE.

Slot mapping sigma(n) = 128*(n%64) + n//64: point n lands at gather-out
[partition n//64, group n%64]; weights use the natural p_norm layout;
output DMA is 48KB contiguous per partition.
"""

import numpy as np

B = 8
NPTS = 8192
C = 128
CC = 64
# (tag, H, W)
SCALES = [("4", 128, 128), ("8", 64, 64), ("16", 32, 32)]

_CACHE = {}


def _build_nc():
    import concourse.bass as bass
    import concourse.bacc as bacc
    import concourse.mybir as mybir
    from concourse.tile import TileContext

    f32 = mybir.dt.float32
    i32 = mybir.dt.int32
    i16 = mybir.dt.int16
    AF = mybir.ActivationFunctionType
    OP = mybir.AluOpType

    nc = bacc.Bacc(dynamic_dma_scratch_size=32768)

    P_dram = {}
    Wt_dram = {}
    br_dram = {}
    for tag, H, W in SCALES:
        P_dram[tag] = nc.dram_tensor(f"P{tag}", [C, H, W], f32, kind="ExternalInput")
        Wt_dram[tag] = nc.dram_tensor(f"W{tag}T", [C, CC], f32, kind="ExternalInput")
        br_dram[tag] = nc.dram_tensor(f"b{tag}r", [128, CC], f32, kind="ExternalInput")
    pn_dram = nc.dram_tensor("p_norm", [NPTS, 2], f32, kind="ExternalInput")
    pnA_dram = nc.dram_tensor("p_norm_A", [128, 8, 64, 2], f32, kind="ExternalInput")
    out_dram = nc.dram_tensor("out", [NPTS, 3 * CC], f32, kind="ExternalOutput")

    with TileContext(nc) as tc:
        with (
            tc.tile_pool(name="const", bufs=1) as pC,
            tc.tile_pool(name="pts", bufs=1) as pT,
            tc.tile_pool(name="p512", bufs=2) as p5,
            tc.tile_pool(name="pnat", bufs=2) as pN,
            tc.tile_pool(name="pchunk", bufs=2) as pP,
            tc.tile_pool(name="qt", bufs=2) as pQT,
            tc.tile_pool(name="vg", bufs=2) as pV,
            tc.tile_pool(name="outp", bufs=1) as pO,
            tc.tile_pool(name="ps", bufs=4, space="PSUM") as pps,
            tc.tile_pool(name="dram", bufs=1, space="DRAM") as pD,
        ):
            # ---- constants / p_norm loads ----
            wt = {}
            br = {}
            for tag, H, W in SCALES:
                wt[tag] = pC.tile([C, CC], f32, tag=f"wt{tag}", name=f"wt{tag}")
                nc.sync.dma_start(out=wt[tag][:], in_=Wt_dram[tag][:])
                br[tag] = pC.tile([128, CC], f32, tag=f"br{tag}", name=f"br{tag}")
                nc.sync.dma_start(out=br[tag][:], in_=br_dram[tag][:])

            # natural layout: partition q holds points 64q..64q+64
            pn_nat = pC.tile([128, 64, 2], f32, tag="pn_nat", name="pn_nat")
            nc.sync.dma_start(
                out=pn_nat[:], in_=pn_dram[:].rearrange("(q f) t -> q f t", q=128)
            )
            # A-layout (for gather idx): partition 16r+qh holds n=1024a+64qh+f
            # at free (a, f); replicated 8x across partition groups.
            pn_A = pC.tile([128, 8, 64, 2], f32, tag="pn_A", name="pn_A")
            nc.sync.dma_start(out=pn_A[:], in_=pnA_dram[:])

            zrow = pC.tile([1, CC], f32, tag="zrow", name="zrow")
            nc.vector.memset(zrow[:], 0.0)

            out_sb = pO.tile([128, 64, 3 * CC], f32, tag="OUT", name="OUT")

            def floor_bp(pool, z, nm, shape):
                """both-proof floor: works whether HW f32->i32 converts by
                truncation or round-to-nearest. z >= 0 required."""
                ti = pool.tile(shape, i32, tag="ti", name="ti")
                nc.vector.tensor_copy(out=ti[:], in_=z[:])
                tf = pool.tile(shape, f32, tag="tf", name="tf")
                nc.vector.tensor_copy(out=tf[:], in_=ti[:])
                corr = pool.tile(shape, f32, tag="corr", name="corr")
                nc.vector.tensor_tensor(out=corr[:], in0=tf[:], in1=z[:], op=OP.is_gt)
                fl = pool.tile(shape, f32, tag=f"fl{nm}", name=f"fl{nm}")
                nc.vector.tensor_tensor(
                    out=fl[:], in0=tf[:], in1=corr[:], op=OP.subtract
                )
                return fl

            # ====== phase 1: all point math (idx + weights, 3 scales) ======
            idx16 = {}
            WAB = {}
            for tag, H, W in SCALES:
                # ---- gather indices (A-layout [128, (a=8, f=64)]) ----
                zx = p5.tile([128, 512], f32, tag="zx", name="zx")
                nc.scalar.activation(
                    out=zx[:].rearrange("p (a f) -> p a f", a=8),
                    in_=pn_A[:, :, :, 0],
                    func=AF.Copy,
                    scale=W / 2.0,
                    bias=W / 2.0 + 0.5,
                )
                zy = p5.tile([128, 512], f32, tag="zy", name="zy")
                nc.scalar.activation(
                    out=zy[:].rearrange("p (a f) -> p a f", a=8),
                    in_=pn_A[:, :, :, 1],
                    func=AF.Copy,
                    scale=H / 2.0,
                    bias=H / 2.0 + 0.5,
                )
                xb = floor_bp(p5, zx, "x", [128, 512])  # = floor(x)+1 in [0, W]
                ybf = floor_bp(p5, zy, "y", [128, 512])  # = floor(y)+1 in [0, H]
                yA = p5.tile([128, 512], f32, tag="yA", name="yA")
                nc.vector.tensor_scalar(
                    out=yA[:], in0=ybf[:], scalar1=-1.0, scalar2=0.0,
                    op0=OP.add, op1=OP.max,
                )
                yB = p5.tile([128, 512], f32, tag="yB", name="yB")
                nc.vector.tensor_scalar(
                    out=yB[:], in0=ybf[:], scalar1=float(H - 1), scalar2=None,
                    op0=OP.min,
                )
                for nm, yy in (("A", yA), ("B", yB)):
                    idxf = p5.tile([128, 512], f32, tag=f"idxf{nm}", name=f"idxf{nm}")
                    nc.vector.scalar_tensor_tensor(
                        out=idxf[:], in0=yy[:], scalar=float(W), in1=xb[:],
                        op0=OP.mult, op1=OP.add,
                    )
                    ix = pT.tile(
                        [128, 512], i16, tag=f"idx{nm}{tag}", name=f"idx{nm}{tag}"
                    )
                    # reorder (a, f) -> s = 8f + a while casting
                    nc.vector.tensor_copy(
                        out=ix[:].rearrange("p (f a) -> p f a", a=8),
                        in_=idxf[:].rearrange("p (a f) -> p f a", a=8),
                    )
                    idx16[nm, tag] = ix

                # ---- weights (natural layout [128, 64]) ----
                zxn = pN.tile([128, 64], f32, tag="zxn", name="zxn")
                nc.scalar.activation(
                    out=zxn[:], in_=pn_nat[:, :, 0], func=AF.Copy,
                    scale=W / 2.0, bias=W / 2.0 + 0.5,
                )
                zyn = pN.tile([128, 64], f32, tag="zyn", name="zyn")
                nc.scalar.activation(
                    out=zyn[:], in_=pn_nat[:, :, 1], func=AF.Copy,
                    scale=H / 2.0, bias=H / 2.0 + 0.5,
                )
                xbn = floor_bp(pN, zxn, "nx", [128, 64])
                ybn = floor_bp(pN, zyn, "ny", [128, 64])

                def frac_weights(z, fl, lim):
                    # w1 = frac = z - floor(z); w0 = 1 - w1
                    # valid0 = z >= 1; valid1 = z < lim
                    w1 = pN.tile([128, 64], f32, tag="w1", name="w1")
                    nc.vector.tensor_tensor(
                        out=w1[:], in0=z[:], in1=fl[:], op=OP.subtract
                    )
                    w0 = pN.tile([128, 64], f32, tag="w0", name="w0")
                    nc.vector.tensor_scalar(
                        out=w0[:], in0=w1[:], scalar1=-1.0, scalar2=1.0,
                        op0=OP.mult, op1=OP.add,
                    )
                    v0 = pN.tile([128, 64], f32, tag="v0", name="v0")
                    nc.vector.tensor_scalar(
                        out=v0[:], in0=z[:], scalar1=1.0, scalar2=None, op0=OP.is_ge
                    )
                    v1 = pN.tile([128, 64], f32, tag="v1", name="v1")
                    nc.vector.tensor_scalar(
                        out=v1[:], in0=z[:], scalar1=float(lim), scalar2=None,
                        op0=OP.is_lt,
                    )
                    w0v = pN.tile([128, 64], f32, tag="w0v", name="w0v")
                    nc.vector.tensor_tensor(out=w0v[:], in0=w0[:], in1=v0[:], op=OP.mult)
                    w1v = pN.tile([128, 64], f32, tag="w1v", name="w1v")
                    nc.vector.tensor_tensor(out=w1v[:], in0=w1[:], in1=v1[:], op=OP.mult)
                    return w0v, w1v

                wx0, wx1 = frac_weights(zxn, xbn, W)
                wyA, wyB = frac_weights(zyn, ybn, H)

                for nm, wy in (("A", wyA), ("B", wyB)):
                    Wt_ = pT.tile(
                        [128, 64, 2], f32, tag=f"W{nm}{tag}", name=f"W{nm}{tag}"
                    )
                    nc.vector.tensor_tensor(
                        out=Wt_[:, :, 0], in0=wy[:], in1=wx0[:], op=OP.mult
                    )
                    nc.vector.tensor_tensor(
                        out=Wt_[:, :, 1], in0=wy[:], in1=wx1[:], op=OP.mult
                    )
                    WAB[nm, tag] = Wt_

            # ====== phase 2: per scale conv -> table -> gather -> combine ======
            for si, (tag, H, W) in enumerate(SCALES):
                HW = H * W
                T = pD.tile([HW + 2, CC], f32, tag=f"T{tag}", name=f"T{tag}")
                # zero the guard rows
                nc.sync.dma_start(out=T[0:1, :], in_=zrow[:])
                nc.sync.dma_start(out=T[HW + 1 : HW + 2, :], in_=zrow[:])

                # ---- conv: chunks of <=2048 pixels ----
                Pflat = P_dram[tag][:].rearrange("c h w -> c (h w)")
                QS = min(1024, HW)
                J = QS // 128  # pixels per partition within a chunk
                for q0 in range(0, HW, QS):
                    Pq = pP.tile([C, QS], f32, tag="P", name="P")
                    nc.sync.dma_start(out=Pq[:], in_=Pflat[:, q0 : q0 + QS])
                    PqV = Pq[:].rearrange("c (p j) -> c p j", j=J)
                    Tq = T[1 + q0 : 1 + q0 + QS, :].rearrange("(p j) c -> p j c", j=J)
                    for jb in range(0, J, 8):
                        nj = min(8, J - jb)
                        ps = pps.tile([128, 512], f32, tag="ps", name="ps")
                        for k in range(nj):
                            nc.tensor.matmul(
                                out=ps[:, CC * k : CC * (k + 1)],
                                lhsT=PqV[:, :, jb + k],
                                rhs=wt[tag][:],
                                start=True,
                                stop=True,
                            )
                        qt = pQT.tile([128, 8, CC], f32, tag="qt", name="qt")
                        nc.vector.scalar_tensor_tensor(
                            out=qt[:, :nj],
                            in0=ps[:, : CC * nj].rearrange("p (g x) -> p g x", x=CC),
                            scalar=0.0,
                            in1=br[tag][:, None, :].to_broadcast([128, nj, CC]),
                            op0=OP.bypass,
                            op1=OP.add,
                        )
                        nc.sync.dma_start(out=Tq[:, jb : jb + nj, :], in_=qt[:, :nj])

                # ---- pair gathers + combine ----
                t_ap = T[:]
                pair_ap = bass.AP(t_ap.tensor, t_ap.offset, [[CC, HW + 1], [1, 2 * CC]])
                V = {}
                GCH = 1024  # dma_gather is stable up to 1024 idxs per call
                for nm in ("A", "B"):
                    Vt = pV.tile([128, 64, 2, CC], f32, tag="V", name="V")
                    for c in range(0, NPTS, GCH):
                        g0 = c // 128
                        g1 = (c + GCH) // 128
                        nc.gpsimd.dma_gather(
                            out_ap=Vt[:, g0:g1].rearrange("p g t c -> p g (t c)"),
                            in_ap=pair_ap,
                            idxs_ap=idx16[nm, tag][:, c // 16 : (c + GCH) // 16],
                            num_idxs=GCH,
                            num_idxs_reg=GCH,
                            elem_size=2 * CC,
                            elem_step=CC,
                        )
                    V[nm] = Vt
                for nm in ("A", "B"):
                    Vt = V[nm]
                    # in-place weight multiply (per-point weights bcast over c)
                    nc.vector.tensor_tensor(
                        out=Vt[:],
                        in0=Vt[:],
                        in1=WAB[nm, tag][:, :, :, None].to_broadcast([128, 64, 2, CC]),
                        op=OP.mult,
                    )
                    # fold the two x-taps
                    nc.vector.tensor_tensor(
                        out=Vt[:, :, 0, :],
                        in0=Vt[:, :, 0, :],
                        in1=Vt[:, :, 1, :],
                        op=OP.add,
                    )
                nc.vector.tensor_tensor(
                    out=out_sb[:, :, CC * si : CC * (si + 1)],
                    in0=V["A"][:, :, 0, :],
                    in1=V["B"][:, :, 0, :],
                    op=OP.add,
                )

            nc.sync.dma_start(
                out=out_dram[:].rearrange("(q f) c -> q f c", q=128),
                in_=out_sb[:],
            )

    nc.finalize()
    return nc


def _get_nc():
    if "nc" not in _CACHE:
        _CACHE["nc"] = _build_nc()
    return _CACHE["nc"]


def _make_in_maps(inputs):
    f32 = np.float32
    shared = {}
    for tag, H, W in SCALES:
        shared[f"W{tag}T"] = np.ascontiguousarray(
            np.asarray(inputs[f"W{tag}"], dtype=f32).T
        )
        shared[f"b{tag}r"] = np.ascontiguousarray(
            np.tile(np.asarray(inputs[f"b{tag}"], dtype=f32)[None, :], (128, 1))
        )
    in_maps = []
    for b in range(B):
        m = dict(shared)
        for tag, H, W in SCALES:
            m[f"P{tag}"] = np.ascontiguousarray(
                np.asarray(inputs[f"P{tag}"][b], dtype=f32)
            )
        pnb = np.asarray(inputs["p_norm"][b], dtype=f32)
        m["p_norm"] = np.ascontiguousarray(pnb)
        # A-layout: partition 16r+qh holds n=1024a+64qh+f at free (a,f,xy)
        pA = pnb.reshape(8, 16, 64, 2).transpose(1, 0, 2, 3)  # [qh, a, f, t]
        m["p_norm_A"] = np.ascontiguousarray(np.tile(pA, (8, 1, 1, 1)))
        in_maps.append(m)
    return in_maps


def kernel(**inputs):
    from concourse.bass_utils import run_bass_kernel_spmd

    nc = _get_nc()
    in_maps = _make_in_maps(inputs)
    res = run_bass_kernel_spmd(nc, in_maps, core_ids=list(range(B)))
    return np.stack([r["out"] for r in res.results])
